# revision 41
# baseline (speedup 1.0000x reference)
"""Trainium2 Bass kernel for nn_AttentionBlock (GroupNorm + 1x1-conv QKV +
multi-head attention + 1x1-conv proj + residual).

Contract: kernel(**inputs) takes the FULL unsharded inputs (numpy) and
returns the FULL output.  Internally shards data-parallel over batch across
8 NeuronCores (2 samples per core).

v3 design notes (baseline 558-619us, v2 348us):
  - Attention runs one head at a time, software-pipelined so ScalarE's exp
    stream is saturated: QK scores land as [128 s, 1024 t] f32 in a 2-bank
    PSUM tile (one [128,1024] exp per (head, s-chunk) amortizes the
    ~350-cycle ACT fixed cost); score tiles round-robin through 2 buffers so
    QK(m+1) overlaps exp(m).
  - AV accumulates [v^T | 1] @ e into a [65, 1024] PSUM tile per head; the
    extra ones-column emits the softmax normalizer Z as row 64.
  - 1/Z via reciprocal_approx_fast (single-pass custom DVE op) after a hop
    through SBUF; broadcast via GpSimd; one [64, 1024] multiply per head.
  - v-bias and proj-bias fold into b_eff = proj_w @ v_bias + proj_b
    (softmax rows sum to 1), added into the residual input on DVE.
  - HAM throttle management (the big v3 win): the PE clock halves (K=4/8)
    after any >3.4us PE-idle window and, once cold, an exp-gated attention
    loop never re-warms.  So: sample 1's GN/V/QKV phases are emitted
    INTERLEAVED into sample 0's attention (the Scalar-bound stretch has PE
    headroom), the V phase precedes QKV (its PSUM evacuations otherwise
    stall attention startup), and x/GN-constant DMAs precede the big weight
    DMAs so compute starts early.
"""

import math
import os

import numpy as np

import concourse.bacc as bacc
import concourse.tile as tile
from concourse import mybir
from concourse.bass_utils import run_bass_kernel_spmd

F32 = mybir.dt.float32
AX = mybir.AxisListType
ALU = mybir.AluOpType
ACT = mybir.ActivationFunctionType

N_CORES = 8
B, C, HH, WW = 16, 512, 32, 32
L = HH * WW            # 1024
BL = B // N_CORES      # batches per core = 2
NH = 8                 # heads
CH = C // NH           # head dim = 64
GROUPS = 32
GS = C // GROUPS       # channels per group = 16
EPS = 1e-5
SCALE2 = 1.0 / math.sqrt(CH)   # combined q*k scale, folded into exp
CT = C // 128          # channel tiles = 4
ST = L // 128          # s-chunks = 8
INV_N = 1.0 / (GS * L)         # 1/16384 for group mean

MM_DT = mybir.dt.bfloat16
# debug bisection: 1=gn, 2=+qkv, 4=full, 5=dump Z
STAGE = int(os.environ.get("KERNEL_STAGE", "4"))

LAST_RESULTS = None  # test harness can read exec_time_ns from here


def _build_program():
    nc = bacc.Bacc("TRN2", target_bir_lowering=False, debug=False,
                   num_devices=N_CORES)

    x_d = nc.dram_tensor("x", [BL, C, L], F32, kind="ExternalInput").ap()
    out_d = nc.dram_tensor("out", [BL, C, L], F32, kind="ExternalOutput").ap()
    wqT_d = nc.dram_tensor("wqkvT", [C, 3 * C], MM_DT, kind="ExternalInput").ap()
    wpT_d = nc.dram_tensor("wprojT", [C, C], MM_DT, kind="ExternalInput").ap()
    nw_d = nc.dram_tensor("norm_w", [C], F32, kind="ExternalInput").ap()
    nb_d = nc.dram_tensor("norm_b", [C], F32, kind="ExternalInput").ap()
    qb_d = nc.dram_tensor("qkv_b", [3 * C], F32, kind="ExternalInput").ap()
    beff_d = nc.dram_tensor("b_eff", [C], F32, kind="ExternalInput").ap()
    sel_d = nc.dram_tensor("sel", [CT, 128, GROUPS], F32, kind="ExternalInput").ap()
    fan_d = nc.dram_tensor("fan", [CT, GROUPS, 128], F32, kind="ExternalInput").ap()

    VW = NH * (CH + 1)         # 520: per-s-chunk v^T row width (8 heads x 65)

    with tile.TileContext(nc) as tc:
        with (
            tc.tile_pool(name="wgt", bufs=1) as wgt,          # persistent
            tc.tile_pool(name="xs", bufs=2 * CT) as xs_p,     # raw x tiles
            tc.tile_pool(name="xn", bufs=2 * CT) as xn_p,     # normalized x
            tc.tile_pool(name="qk", bufs=4 * CT) as qk_p,     # q,k both samples
            tc.tile_pool(name="ew", bufs=4) as ew_p,          # exp(wT) chunks
            tc.tile_pool(name="apool", bufs=2 * CT) as a_p,   # attention out
            tc.tile_pool(name="zz", bufs=2) as z_p,           # Z rows / 1/Z
            tc.tile_pool(name="zb", bufs=2) as zb_p,          # 1/Z broadcast
            tc.tile_pool(name="outs", bufs=2) as out_p,       # residual out
            tc.tile_pool(name="tiny", bufs=8) as tiny,        # gn stats etc.
            tc.tile_pool(name="scr", bufs=2) as scr_p,        # bn stats
            tc.tile_pool(name="ps", bufs=2, space="PSUM") as ps_p,  # shared
        ):
            # ---------------- x(b=0) + GN constants first ----------------
            st8 = {}   # per-sample dicts of live tiles
            for b in range(BL):
                st8[b] = {}

            def load_x(b):
                xs = []
                for i in range(CT):
                    t = xs_p.tile([128, L], F32, tag="xs", name=f"xs{b}_{i}")
                    nc.sync.dma_start(t[:], x_d[b, 128 * i:128 * (i + 1), :])
                    xs.append(t)
                st8[b]["xs"] = xs

            load_x(0)

            sel = []
            fan = []
            nw = []
            nb = []
            beff = []
            for i in range(CT):
                s = wgt.tile([128, GROUPS], F32, tag=f"sel{i}")
                nc.sync.dma_start(s[:], sel_d[i, :, :])
                sel.append(s)
                f = wgt.tile([GROUPS, 128], F32, tag=f"fan{i}")
                nc.sync.dma_start(f[:], fan_d[i, :, :])
                fan.append(f)
                t = wgt.tile([128, 1], F32, tag=f"nw{i}")
                nc.sync.dma_start(t[:], nw_d[128 * i:128 * (i + 1)].rearrange("(p a) -> p a", a=1))
                nw.append(t)
                t = wgt.tile([128, 1], F32, tag=f"nb{i}")
                nc.sync.dma_start(t[:], nb_d[128 * i:128 * (i + 1)].rearrange("(p a) -> p a", a=1))
                nb.append(t)
                t = wgt.tile([128, 1], F32, tag=f"beff{i}")
                nc.sync.dma_start(t[:], beff_d[128 * i:128 * (i + 1)].rearrange("(p a) -> p a", a=1))
                beff.append(t)
            eps_t = wgt.tile([GROUPS, 1], F32, tag="eps")
            nc.gpsimd.memset(eps_t[:], EPS)
            load_x(1)

            # ---------------- big weights ----------------
            wq = []   # qkv_w^T tiles [128 c', 1536 o]
            wp = []   # proj_w^T tiles [128 c', 512 o]
            for i in range(CT):
                w = wgt.tile([128, 3 * C], MM_DT, tag=f"wq{i}")
                nc.sync.dma_start(w[:], wqT_d[128 * i:128 * (i + 1), :])
                wq.append(w)
                w = wgt.tile([128, C], MM_DT, tag=f"wp{i}")
                nc.sync.dma_start(w[:], wpT_d[128 * i:128 * (i + 1), :])
                wp.append(w)
            qb_qk = []
            for j in range(2 * CT):
                t = wgt.tile([128, 1], F32, tag=f"qb{j}")
                nc.sync.dma_start(t[:], qb_d[128 * j:128 * (j + 1)].rearrange("(p a) -> p a", a=1))
                qb_qk.append(t)
            # persistent v^T stores, one per sample; ones-columns are memset
            # once (evacs only write cols 0:64 of each 65-block)
            vt_all = []
            for p in range(BL):
                v = wgt.tile([128, ST * VW], MM_DT, tag=f"vt{p}")
                nc.gpsimd.memset(v[:], 1.0)
                vt_all.append(v)

            # ---------------- phase emitters ----------------
            def gn_stats(b):
                xs = st8[b]["xs"]
                stats = []
                for i in range(CT):
                    bns = scr_p.tile([128, 2, 6], F32, tag="bns", name=f"bns{b}_{i}")
                    xv = xs[i][:].rearrange("p (s f) -> p s f", f=512)
                    for sgi in range(2):
                        nc.vector.bn_stats(bns[:, sgi, :], xv[:, sgi, :])
                    mv = tiny.tile([128, 2], F32, tag="mv", name=f"mv{b}_{i}")
                    nc.vector.bn_aggr(mv[:], bns[:])
                    st = tiny.tile([128, 2], F32, tag="stats", name=f"st{b}_{i}")
                    # sum = L*mean ; sumsq = L*(var + mean^2)
                    nc.vector.tensor_scalar_mul(st[:, 0:1], mv[:, 0:1], float(L))
                    m2 = tiny.tile([128, 2], F32, tag="m2", name=f"m2{b}_{i}")
                    nc.vector.tensor_mul(m2[:, 0:1], mv[:, 0:1], mv[:, 0:1])
                    nc.vector.tensor_add(m2[:, 1:2], mv[:, 1:2], m2[:, 0:1])
                    nc.vector.tensor_scalar_mul(st[:, 1:2], m2[:, 1:2], float(L))
                    stats.append(st)
                st8[b]["stats"] = stats

            def gn(b):
                xs = st8[b]["xs"]
                stats = st8[b]["stats"]
                pg = ps_p.tile([128, 1024], F32, tag="mm", name=f"pg{b}")
                for i in range(CT):
                    nc.tensor.matmul(pg[0:GROUPS, 0:2], sel[i][:, :], stats[i][:, :],
                                     start=(i == 0), stop=(i == CT - 1))
                # group quantities: mean, E[x2], mean^2, var, std, rstd
                gq = tiny.tile([GROUPS, 8], F32, tag="gq", name=f"gq{b}")
                nc.vector.tensor_scalar_mul(gq[:, 0:1], pg[0:GROUPS, 0:1], INV_N)
                nc.vector.tensor_scalar_mul(gq[:, 1:2], pg[0:GROUPS, 1:2], INV_N)
                nc.vector.tensor_mul(gq[:, 2:3], gq[:, 0:1], gq[:, 0:1])
                nc.vector.tensor_sub(gq[:, 3:4], gq[:, 1:2], gq[:, 2:3])
                nc.scalar.activation(gq[:, 4:5], gq[:, 3:4], ACT.Sqrt,
                                     bias=eps_t[:])
                nc.vector.reciprocal(gq[:, 5:6], gq[:, 4:5])
                nc.vector.tensor_mul(gq[:, 6:7], gq[:, 0:1], gq[:, 5:6])
                # fan out to channels, make per-channel scale/bias
                xn = []
                for i in range(CT):
                    pf = ps_p.tile([128, 1024], F32, tag="mm", name=f"pf{b}_{i}")
                    nc.tensor.matmul(pf[0:128, 0:2], fan[i][:, :], gq[:, 5:7],
                                     start=True, stop=True)
                    scb = tiny.tile([128, 3], F32, tag="scb", name=f"scb{b}_{i}")
                    nc.vector.tensor_mul(scb[:, 0:1], pf[0:128, 0:1], nw[i][:])
                    nc.vector.tensor_mul(scb[:, 1:2], pf[0:128, 1:2], nw[i][:])
                    nc.vector.tensor_sub(scb[:, 2:3], nb[i][:], scb[:, 1:2])
                    t = xn_p.tile([128, L], MM_DT, tag="xn", name=f"xn{b}_{i}")
                    nc.vector.tensor_scalar(t[:], xs[i][:], scb[:, 0:1], scb[:, 2:3],
                                            op0=ALU.mult, op1=ALU.add)
                    xn.append(t)
                    # xs now only feeds the residual: fold b_eff in
                    nc.vector.tensor_scalar_add(xs[i][:], xs[i][:], beff[i][:])
                st8[b]["xn"] = xn

            def _v_chain(b, mp, hf, pv):
                xn = st8[b]["xn"]
                m = 2 * mp + hf
                nsl = slice(512 * hf, 512 * (hf + 1))
                for i in range(CT):
                    nc.tensor.matmul(
                        pv[:, nsl],
                        xn[i][:, 128 * m:128 * (m + 1)],
                        wq[i][:, 2 * C:3 * C],
                        start=(i == 0), stop=(i == CT - 1))

            def _v_evac(b, mp, pv):
                vt = vt_all[b]
                vtv = vt[:].rearrange("p (m h x) -> p m h x", h=NH, x=CH + 1)
                src = pv[:, :].rearrange("p (m h x) -> p m h x", h=NH, x=CH)
                nc.vector.tensor_copy(vtv[:, 2 * mp:2 * mp + 2, :, 0:CH], src)

            def vphase(b, mp, tag="mm"):
                # v^T [s, c] into persistent vt tile, two s-chunks per evac
                pv = ps_p.tile([128, 1024], F32, tag=tag, name=f"pv{b}_{mp}")
                _v_chain(b, mp, 0, pv)
                _v_chain(b, mp, 1, pv)
                _v_evac(b, mp, pv)

            def v_inserts(b, mp):
                # two 4-matmul half-chains: each fits inside the QK pipeline's
                # completion margin, so the exp stream barely stalls
                state = {}

                def part(hf):
                    def go():
                        if "pv" not in state:
                            state["pv"] = ps_p.tile([128, 1024], F32, tag="pa",
                                                    name=f"pv{b}_{mp}")
                        _v_chain(b, mp, hf, state["pv"])
                        if hf == 1:
                            _v_evac(b, mp, state["pv"])
                    return go
                return {2: part(0), 5: part(1)}

            def _qkv_chain(b, j, n, pq):
                xn = st8[b]["xn"]
                nsl = slice(512 * n, 512 * (n + 1))
                for i in range(CT):
                    nc.tensor.matmul(
                        pq[:, nsl],
                        wq[i][:, 128 * j:128 * (j + 1)],
                        xn[i][:, nsl],
                        start=(i == 0), stop=(i == CT - 1))

            def _qkv_evac(b, j, pq):
                qk = st8[b].setdefault("qk", [None] * (2 * CT))
                t = qk_p.tile([128, L], MM_DT, tag="qk", name=f"qk{b}_{j}")
                nc.vector.tensor_scalar_add(t[:], pq[:, :], qb_qk[j][:])
                qk[j] = t

            def qkv(b, j, tag="mm"):
                pq = ps_p.tile([128, 1024], F32, tag=tag, name=f"pq{b}_{j}")
                _qkv_chain(b, j, 0, pq)
                _qkv_chain(b, j, 1, pq)
                _qkv_evac(b, j, pq)

            def qkv_inserts(b, j):
                state = {}

                def part(n):
                    def go():
                        if "pq" not in state:
                            state["pq"] = ps_p.tile([128, 1024], F32, tag="pa",
                                                    name=f"pq{b}_{j}")
                        _qkv_chain(b, j, n, state["pq"])
                        if n == 1:
                            _qkv_evac(b, j, state["pq"])
                    return go
                return {2: part(0), 5: part(1)}

            def attn_head(b, h, insert_after_m=None, insert_end=None):
                qk = st8[b]["qk"]
                vt = vt_all[b]
                a_tiles = st8[b].setdefault("a", [None] * CT)
                jq = h // 2
                rq = slice(64 * (h % 2), 64 * (h % 2) + 64)
                tp = (64 * (h % 2), 0)
                q_t = qk[jq]
                k_t = qk[CT + jq]
                pa_t = ps_p.tile([CH + 1, 1024], F32, tag="pa",
                                 name=f"pa{b}_{h}")
                prev = None
                for m in range(ST):
                    msl = slice(128 * m, 128 * (m + 1))
                    kq_t = ps_p.tile([128, 1024], F32, tag="mm",
                                     name=f"kq{b}_{h}_{m}")
                    for n in range(2):
                        nsl = slice(512 * n, 512 * (n + 1))
                        nc.tensor.matmul(kq_t[:, nsl],
                                         k_t[rq, msl],
                                         q_t[rq, nsl],
                                         start=True, stop=True,
                                         tile_position=tp)
                    e_t = ew_p.tile([128, L], MM_DT, tag="ew",
                                    name=f"ew{b}_{h}_{m}")
                    nc.scalar.activation(e_t[:], kq_t[:, :], ACT.Exp,
                                         scale=SCALE2)
                    if prev is not None:
                        pm, pe = prev
                        lhs = vt[:, (pm * NH + h) * (CH + 1):
                                 (pm * NH + h + 1) * (CH + 1)]
                        for n in range(2):
                            nsl = slice(512 * n, 512 * (n + 1))
                            nc.tensor.matmul(pa_t[:, nsl], lhs, pe[:, nsl],
                                             start=(pm == 0), stop=False)
                    prev = (m, e_t)
                    if insert_after_m and m in insert_after_m:
                        insert_after_m[m]()
                pm, pe = prev
                lhs = vt[:, (pm * NH + h) * (CH + 1):
                         (pm * NH + h + 1) * (CH + 1)]
                for n in range(2):
                    nsl = slice(512 * n, 512 * (n + 1))
                    nc.tensor.matmul(pa_t[:, nsl], lhs, pe[:, nsl],
                                     start=False, stop=True)
                # normalize: 1/Z (fast approx) -> broadcast -> multiply
                # (reciprocal_approx_fast misreads PSUM in this context —
                # hop through SBUF partition 0 first)
                zt = z_p.tile([1, L], F32, tag="zt", name=f"zt{b}_{h}")
                nc.vector.tensor_copy(zt[:], pa_t[CH:CH + 1, :])
                rz = z_p.tile([1, L], F32, tag="rz", name=f"rz{b}_{h}")
                nc.vector.reciprocal_approx_fast(rz[:], zt[:])
                rzb = zb_p.tile([CH, L], F32, tag="zb", name=f"zb{b}_{h}")
                nc.gpsimd.partition_broadcast(rzb[:], rz[:])
                if h % 2 == 0:
                    a_tiles[h // 2] = a_p.tile([128, L], MM_DT, tag="a",
                                               name=f"a{b}_{h // 2}")
                rows = slice(CH * (h % 2), CH * (h % 2) + CH)
                nc.vector.tensor_mul(a_tiles[h // 2][rows, :],
                                     pa_t[0:CH, :], rzb[:])
                if insert_end:
                    insert_end()

            def proj(b, js=range(CT)):
                xs = st8[b]["xs"]
                a_tiles = st8[b]["a"]
                for j in js:
                    pp = ps_p.tile([128, 1024], F32, tag="mm", name=f"pp{b}_{j}")
                    for n in range(2):
                        nsl = slice(512 * n, 512 * (n + 1))
                        for i in range(CT):
                            nc.tensor.matmul(
                                pp[:, nsl],
                                wp[i][:, 128 * j:128 * (j + 1)],
                                a_tiles[i][:, nsl],
                                start=(i == 0), stop=(i == CT - 1))
                    o_t = out_p.tile([128, L], F32, tag="o", name=f"o{b}_{j}")
                    nc.vector.tensor_add(o_t[:], pp[:, :], xs[j][:])
                    nc.sync.dma_start(out_d[b, 128 * j:128 * (j + 1), :], o_t[:])

            def dump_stage(b, tiles):
                for i in range(CT):
                    nc.sync.dma_start(out_d[b, 128 * i:128 * (i + 1), 0:512],
                                      tiles[i][:].bitcast(F32))

            def warmup():
                # ~4us of full-array matmuls as soon as the weights land, so
                # the HAM clock gate reaches 8/8 before the real phases
                dummy = ps_p.tile([128, 1024], F32, tag="mm", name="dummy")
                for k in range(18):
                    nc.tensor.matmul(dummy[:, 0:512], wq[0][:, 0:128],
                                     wq[0][:, 0:512],
                                     start=(k == 0), stop=(k == 17))

            # ---------------- emission schedule ----------------
            # (gn_stats(1) must come AFTER gn(0)'s DVE ops: the DVE queue is
            # in-order and gn_stats(1) blocks on the x(1) DMA)
            gn_stats(0)
            warmup()
            gn(0)
            gn_stats(1)
            if STAGE == 1:
                dump_stage(0, st8[0]["xn"])
                gn(1)
                dump_stage(1, st8[1]["xn"])
            elif STAGE == 2:
                for mp in range(4):
                    vphase(0, mp)
                for j in range(8):
                    qkv(0, j)
                dump_stage(0, st8[0]["qk"][0:CT])
                gn(1)
                for mp in range(4):
                    vphase(1, mp)
                for j in range(8):
                    qkv(1, j)
                dump_stage(1, st8[1]["qk"][0:CT])
            else:
                # Sequential dense blocks; every attention block is entered
                # with the PE warm (preceded by full-array matmul phases) and
                # is never PE-idle >~3.4us inside, so the HAM clock gate
                # stays at 8/8.  Both GN phases run up-front (all Sqrt
                # activations precede the first Exp: one table load each).
                # Sample 1's V/QKV groups drip into attention as 4-matmul
                # HALF-chains at m=2/m=5 (each fits the QK pipeline's
                # completion margin, so the exp stream barely stalls),
                # through the pa tag so the kq round-robin is untouched.
                gn(1)
                for mp in range(4):
                    vphase(0, mp)
                for j in range(8):
                    qkv(0, j)
                ins0 = {1: v_inserts(1, 0), 2: v_inserts(1, 1),
                        3: v_inserts(1, 2), 4: v_inserts(1, 3),
                        5: qkv_inserts(1, 0), 6: qkv_inserts(1, 4)}
                for h in range(NH):
                    attn_head(0, h, insert_after_m=ins0.get(h))
                proj(0, js=(0, 1, 2))
                # remaining sample-1 q/k blocks drip in two heads ahead of
                # first use; proj(0)'s last block densifies the A(1) ramp
                ins1 = {0: {1: (lambda: proj(0, js=(3,))), **qkv_inserts(1, 1)},
                        1: qkv_inserts(1, 5), 2: qkv_inserts(1, 2),
                        3: qkv_inserts(1, 6), 4: qkv_inserts(1, 3),
                        5: qkv_inserts(1, 7)}
                for h in range(NH):
                    attn_head(1, h, insert_after_m=ins1.get(h))
                proj(1)

    nc.compile()
    return nc


_prog_cache = {}


def _get_program():
    if "prog" not in _prog_cache:
        _prog_cache["prog"] = _build_program()
    return _prog_cache["prog"]


def _host_constants():
    # group selector: sel[i][p, g] = 1 where global group of (tile i, part p)
    # is g;  fan[i][g, p] = same, transposed (for the fan-out matmul lhsT).
    sel = np.zeros((CT, 128, GROUPS), dtype=np.float32)
    fan = np.zeros((CT, GROUPS, 128), dtype=np.float32)
    for i in range(CT):
        for p in range(128):
            g = (128 * i + p) // GS
            sel[i, p, g] = 1.0
            fan[i, g, p] = 1.0
    return sel, fan


def kernel(x, norm_w, norm_b, qkv_w, qkv_b, proj_w, proj_b):
    global LAST_RESULTS
    x = np.ascontiguousarray(np.asarray(x, dtype=np.float32))
    np_mm = mybir.dt.np(MM_DT)
    qkv_w = np.asarray(qkv_w, dtype=np.float32)
    proj_w = np.asarray(proj_w, dtype=np.float32)
    qkv_b = np.ascontiguousarray(np.asarray(qkv_b, dtype=np.float32))
    proj_b = np.ascontiguousarray(np.asarray(proj_b, dtype=np.float32))
    wqkvT = np.ascontiguousarray(qkv_w.T.astype(np_mm))
    wprojT = np.ascontiguousarray(proj_w.T.astype(np_mm))
    # softmax rows sum to 1, so the v-bias contributes exactly
    # proj_w @ v_bias to the proj output; fold it plus proj_b into one
    # per-channel constant added at the residual.
    b_eff = np.ascontiguousarray(
        proj_w @ qkv_b[2 * C:3 * C] + proj_b).astype(np.float32)
    sel, fan = _host_constants()

    xr = x.reshape(B, C, L)
    nc = _get_program()

    common = {
        "wqkvT": wqkvT,
        "wprojT": wprojT,
        "norm_w": np.ascontiguousarray(norm_w, dtype=np.float32),
        "norm_b": np.ascontiguousarray(norm_b, dtype=np.float32),
        "qkv_b": qkv_b,
        "b_eff": b_eff,
        "sel": sel,
        "fan": fan,
    }
    in_maps = []
    for c in range(N_CORES):
        m = dict(common)
        m["x"] = np.ascontiguousarray(xr[BL * c:BL * (c + 1)])
        in_maps.append(m)

    trace = os.environ.get("KERNEL_TRACE", "0") == "1"
    kwargs = {}
    if trace:
        kwargs = dict(trace=True, trace_cores=[0])
    res = run_bass_kernel_spmd(nc, in_maps, core_ids=list(range(N_CORES)),
                               **kwargs)
    LAST_RESULTS = res
    out = np.concatenate([res.results[c]["out"] for c in range(N_CORES)], axis=0)
    return out.reshape(B, C, HH, WW)


# revision 42
# speedup vs baseline: 1.0534x; 1.0534x over previous
"""Trainium2 Bass kernel for nn_AttentionBlock (GroupNorm + 1x1-conv QKV +
multi-head attention + 1x1-conv proj + residual).

Contract: kernel(**inputs) takes the FULL unsharded inputs (numpy) and
returns the FULL output.  Internally shards data-parallel over batch across
8 NeuronCores (2 samples per core).

v3 design notes (baseline 558-619us, v2 348us):
  - Attention runs one head at a time, software-pipelined so ScalarE's exp
    stream is saturated: QK scores land as [128 s, 1024 t] f32 in a 2-bank
    PSUM tile (one [128,1024] exp per (head, s-chunk) amortizes the
    ~350-cycle ACT fixed cost); score tiles round-robin through 2 buffers so
    QK(m+1) overlaps exp(m).
  - AV accumulates [v^T | 1] @ e into a [65, 1024] PSUM tile per head; the
    extra ones-column emits the softmax normalizer Z as row 64.
  - 1/Z via reciprocal_approx_fast (single-pass custom DVE op) after a hop
    through SBUF; broadcast via GpSimd; one [64, 1024] multiply per head.
  - v-bias and proj-bias fold into b_eff = proj_w @ v_bias + proj_b
    (softmax rows sum to 1), added into the residual input on DVE.
  - HAM throttle management (the big v3 win): the PE clock halves (K=4/8)
    after any >3.4us PE-idle window and, once cold, an exp-gated attention
    loop never re-warms.  So: sample 1's GN/V/QKV phases are emitted
    INTERLEAVED into sample 0's attention (the Scalar-bound stretch has PE
    headroom), the V phase precedes QKV (its PSUM evacuations otherwise
    stall attention startup), and x/GN-constant DMAs precede the big weight
    DMAs so compute starts early.
"""

import math
import os

import numpy as np

import concourse.bacc as bacc
import concourse.tile as tile
from concourse import mybir
from concourse.bass_utils import run_bass_kernel_spmd

F32 = mybir.dt.float32
AX = mybir.AxisListType
ALU = mybir.AluOpType
ACT = mybir.ActivationFunctionType

N_CORES = 8
B, C, HH, WW = 16, 512, 32, 32
L = HH * WW            # 1024
BL = B // N_CORES      # batches per core = 2
NH = 8                 # heads
CH = C // NH           # head dim = 64
GROUPS = 32
GS = C // GROUPS       # channels per group = 16
EPS = 1e-5
SCALE2 = 1.0 / math.sqrt(CH)   # combined q*k scale, folded into exp
CT = C // 128          # channel tiles = 4
ST = L // 128          # s-chunks = 8
INV_N = 1.0 / (GS * L)         # 1/16384 for group mean

MM_DT = mybir.dt.bfloat16
# debug bisection: 1=gn, 2=+qkv, 4=full, 5=dump Z
STAGE = int(os.environ.get("KERNEL_STAGE", "4"))

LAST_RESULTS = None  # test harness can read exec_time_ns from here


def _build_program():
    nc = bacc.Bacc("TRN2", target_bir_lowering=False, debug=False,
                   num_devices=N_CORES)

    x_d = nc.dram_tensor("x", [BL, C, L], F32, kind="ExternalInput").ap()
    out_d = nc.dram_tensor("out", [BL, C, L], F32, kind="ExternalOutput").ap()
    wqT_d = nc.dram_tensor("wqkvT", [C, 3 * C], MM_DT, kind="ExternalInput").ap()
    wpT_d = nc.dram_tensor("wprojT", [C, C], MM_DT, kind="ExternalInput").ap()
    nw_d = nc.dram_tensor("norm_w", [C], F32, kind="ExternalInput").ap()
    nb_d = nc.dram_tensor("norm_b", [C], F32, kind="ExternalInput").ap()
    qb_d = nc.dram_tensor("qkv_b", [3 * C], F32, kind="ExternalInput").ap()
    beff_d = nc.dram_tensor("b_eff", [C], F32, kind="ExternalInput").ap()
    sel_d = nc.dram_tensor("sel", [CT, 128, GROUPS], F32, kind="ExternalInput").ap()
    fan_d = nc.dram_tensor("fan", [CT, GROUPS, 128], F32, kind="ExternalInput").ap()

    VW = NH * (CH + 1)         # 520: per-s-chunk v^T row width (8 heads x 65)

    with tile.TileContext(nc) as tc:
        with (
            tc.tile_pool(name="wgt", bufs=1) as wgt,          # persistent
            tc.tile_pool(name="xs", bufs=2 * CT) as xs_p,     # raw x tiles
            tc.tile_pool(name="xn", bufs=2 * CT) as xn_p,     # normalized x
            tc.tile_pool(name="qk", bufs=4 * CT) as qk_p,     # q,k both samples
            tc.tile_pool(name="ew", bufs=4) as ew_p,          # exp(wT) chunks
            tc.tile_pool(name="apool", bufs=2 * CT) as a_p,   # attention out
            tc.tile_pool(name="zz", bufs=2) as z_p,           # Z rows / 1/Z
            tc.tile_pool(name="zb", bufs=2) as zb_p,          # 1/Z broadcast
            tc.tile_pool(name="outs", bufs=2) as out_p,       # residual out
            tc.tile_pool(name="tiny", bufs=8) as tiny,        # gn stats etc.
            tc.tile_pool(name="scr", bufs=2) as scr_p,        # bn stats
            tc.tile_pool(name="ps", bufs=2, space="PSUM") as ps_p,  # shared
        ):
            # ---------------- x(b=0) + GN constants first ----------------
            st8 = {}   # per-sample dicts of live tiles
            for b in range(BL):
                st8[b] = {}

            def load_x(b):
                xs = []
                for i in range(CT):
                    t = xs_p.tile([128, L], F32, tag="xs", name=f"xs{b}_{i}")
                    nc.sync.dma_start(t[:], x_d[b, 128 * i:128 * (i + 1), :])
                    xs.append(t)
                st8[b]["xs"] = xs

            load_x(0)

            sel = []
            fan = []
            nw = []
            nb = []
            beff = []
            for i in range(CT):
                s = wgt.tile([128, GROUPS], F32, tag=f"sel{i}")
                nc.sync.dma_start(s[:], sel_d[i, :, :])
                sel.append(s)
                f = wgt.tile([GROUPS, 128], F32, tag=f"fan{i}")
                nc.sync.dma_start(f[:], fan_d[i, :, :])
                fan.append(f)
                t = wgt.tile([128, 1], F32, tag=f"nw{i}")
                nc.sync.dma_start(t[:], nw_d[128 * i:128 * (i + 1)].rearrange("(p a) -> p a", a=1))
                nw.append(t)
                t = wgt.tile([128, 1], F32, tag=f"nb{i}")
                nc.sync.dma_start(t[:], nb_d[128 * i:128 * (i + 1)].rearrange("(p a) -> p a", a=1))
                nb.append(t)
                t = wgt.tile([128, 1], F32, tag=f"beff{i}")
                nc.sync.dma_start(t[:], beff_d[128 * i:128 * (i + 1)].rearrange("(p a) -> p a", a=1))
                beff.append(t)
            eps_t = wgt.tile([GROUPS, 1], F32, tag="eps")
            nc.gpsimd.memset(eps_t[:], EPS)
            load_x(1)

            # ---------------- big weights ----------------
            wq = []   # qkv_w^T tiles [128 c', 1536 o]
            wp = []   # proj_w^T tiles [128 c', 512 o]
            for i in range(CT):
                w = wgt.tile([128, 3 * C], MM_DT, tag=f"wq{i}")
                nc.sync.dma_start(w[:], wqT_d[128 * i:128 * (i + 1), :])
                wq.append(w)
                w = wgt.tile([128, C], MM_DT, tag=f"wp{i}")
                nc.sync.dma_start(w[:], wpT_d[128 * i:128 * (i + 1), :])
                wp.append(w)
            qb_qk = []
            for j in range(2 * CT):
                t = wgt.tile([128, 1], F32, tag=f"qb{j}")
                nc.sync.dma_start(t[:], qb_d[128 * j:128 * (j + 1)].rearrange("(p a) -> p a", a=1))
                qb_qk.append(t)
            # persistent v^T stores, one per sample; ones-columns are memset
            # once (evacs only write cols 0:64 of each 65-block)
            vt_all = []
            for p in range(BL):
                v = wgt.tile([128, ST * VW], MM_DT, tag=f"vt{p}")
                nc.gpsimd.memset(v[:], 1.0)
                vt_all.append(v)

            # ---------------- phase emitters ----------------
            def gn_stats(b):
                xs = st8[b]["xs"]
                stats = []
                for i in range(CT):
                    bns = scr_p.tile([128, 2, 6], F32, tag="bns", name=f"bns{b}_{i}")
                    xv = xs[i][:].rearrange("p (s f) -> p s f", f=512)
                    for sgi in range(2):
                        nc.vector.bn_stats(bns[:, sgi, :], xv[:, sgi, :])
                    mv = tiny.tile([128, 2], F32, tag="mv", name=f"mv{b}_{i}")
                    nc.vector.bn_aggr(mv[:], bns[:])
                    st = tiny.tile([128, 2], F32, tag="stats", name=f"st{b}_{i}")
                    # sum = L*mean ; sumsq = L*(var + mean^2)
                    nc.vector.tensor_scalar_mul(st[:, 0:1], mv[:, 0:1], float(L))
                    m2 = tiny.tile([128, 2], F32, tag="m2", name=f"m2{b}_{i}")
                    nc.vector.tensor_mul(m2[:, 0:1], mv[:, 0:1], mv[:, 0:1])
                    nc.vector.tensor_add(m2[:, 1:2], mv[:, 1:2], m2[:, 0:1])
                    nc.vector.tensor_scalar_mul(st[:, 1:2], m2[:, 1:2], float(L))
                    stats.append(st)
                st8[b]["stats"] = stats

            def gn(b):
                xs = st8[b]["xs"]
                stats = st8[b]["stats"]
                pg = ps_p.tile([128, 1024], F32, tag="mm", name=f"pg{b}")
                for i in range(CT):
                    nc.tensor.matmul(pg[0:GROUPS, 0:2], sel[i][:, :], stats[i][:, :],
                                     start=(i == 0), stop=(i == CT - 1))
                # group quantities: mean, E[x2], mean^2, var, std, rstd
                gq = tiny.tile([GROUPS, 8], F32, tag="gq", name=f"gq{b}")
                nc.vector.tensor_scalar_mul(gq[:, 0:1], pg[0:GROUPS, 0:1], INV_N)
                nc.vector.tensor_scalar_mul(gq[:, 1:2], pg[0:GROUPS, 1:2], INV_N)
                nc.vector.tensor_mul(gq[:, 2:3], gq[:, 0:1], gq[:, 0:1])
                nc.vector.tensor_sub(gq[:, 3:4], gq[:, 1:2], gq[:, 2:3])
                nc.scalar.activation(gq[:, 4:5], gq[:, 3:4], ACT.Sqrt,
                                     bias=eps_t[:])
                nc.vector.reciprocal(gq[:, 5:6], gq[:, 4:5])
                nc.vector.tensor_mul(gq[:, 6:7], gq[:, 0:1], gq[:, 5:6])
                # fan out to channels, make per-channel scale/bias
                xn = []
                for i in range(CT):
                    pf = ps_p.tile([128, 1024], F32, tag="mm", name=f"pf{b}_{i}")
                    nc.tensor.matmul(pf[0:128, 0:2], fan[i][:, :], gq[:, 5:7],
                                     start=True, stop=True)
                    scb = tiny.tile([128, 3], F32, tag="scb", name=f"scb{b}_{i}")
                    nc.vector.tensor_mul(scb[:, 0:1], pf[0:128, 0:1], nw[i][:])
                    nc.vector.tensor_mul(scb[:, 1:2], pf[0:128, 1:2], nw[i][:])
                    nc.vector.tensor_sub(scb[:, 2:3], nb[i][:], scb[:, 1:2])
                    t = xn_p.tile([128, L], MM_DT, tag="xn", name=f"xn{b}_{i}")
                    nc.vector.tensor_scalar(t[:], xs[i][:], scb[:, 0:1], scb[:, 2:3],
                                            op0=ALU.mult, op1=ALU.add)
                    xn.append(t)
                    # xs now only feeds the residual: fold b_eff in
                    nc.vector.tensor_scalar_add(xs[i][:], xs[i][:], beff[i][:])
                st8[b]["xn"] = xn

            def vphase(b, mp, tag="mm"):
                # v^T [s, c] into persistent vt tile, two s-chunks per evac
                xn = st8[b]["xn"]
                vt = vt_all[b]
                vtv = vt[:].rearrange("p (m h x) -> p m h x", h=NH, x=CH + 1)
                pv = ps_p.tile([128, 1024], F32, tag=tag, name=f"pv{b}_{mp}")
                for hf in range(2):
                    m = 2 * mp + hf
                    nsl = slice(512 * hf, 512 * (hf + 1))
                    for i in range(CT):
                        nc.tensor.matmul(
                            pv[:, nsl],
                            xn[i][:, 128 * m:128 * (m + 1)],
                            wq[i][:, 2 * C:3 * C],
                            start=(i == 0), stop=(i == CT - 1))
                src = pv[:, :].rearrange("p (m h x) -> p m h x", h=NH, x=CH)
                nc.vector.tensor_copy(vtv[:, 2 * mp:2 * mp + 2, :, 0:CH], src)

            def qkv(b, j, tag="mm"):
                xn = st8[b]["xn"]
                qk = st8[b].setdefault("qk", [None] * (2 * CT))
                pq = ps_p.tile([128, 1024], F32, tag=tag, name=f"pq{b}_{j}")
                for n in range(2):
                    nsl = slice(512 * n, 512 * (n + 1))
                    for i in range(CT):
                        nc.tensor.matmul(
                            pq[:, nsl],
                            wq[i][:, 128 * j:128 * (j + 1)],
                            xn[i][:, nsl],
                            start=(i == 0), stop=(i == CT - 1))
                t = qk_p.tile([128, L], MM_DT, tag="qk", name=f"qk{b}_{j}")
                nc.vector.tensor_scalar_add(t[:], pq[:, :], qb_qk[j][:])
                qk[j] = t

            def attn_head(b, h, insert_after_m=None, insert_end=None):
                qk = st8[b]["qk"]
                vt = vt_all[b]
                a_tiles = st8[b].setdefault("a", [None] * CT)
                jq = h // 2
                rq = slice(64 * (h % 2), 64 * (h % 2) + 64)
                tp = (64 * (h % 2), 0)
                q_t = qk[jq]
                k_t = qk[CT + jq]
                pa_t = ps_p.tile([CH + 1, 1024], F32, tag="pa",
                                 name=f"pa{b}_{h}")
                prev = None
                for m in range(ST):
                    msl = slice(128 * m, 128 * (m + 1))
                    kq_t = ps_p.tile([128, 1024], F32, tag="mm",
                                     name=f"kq{b}_{h}_{m}")
                    for n in range(2):
                        nsl = slice(512 * n, 512 * (n + 1))
                        nc.tensor.matmul(kq_t[:, nsl],
                                         k_t[rq, msl],
                                         q_t[rq, nsl],
                                         start=True, stop=True,
                                         tile_position=tp)
                    e_t = ew_p.tile([128, L], MM_DT, tag="ew",
                                    name=f"ew{b}_{h}_{m}")
                    nc.scalar.activation(e_t[:], kq_t[:, :], ACT.Exp,
                                         scale=SCALE2)
                    if prev is not None:
                        pm, pe = prev
                        lhs = vt[:, (pm * NH + h) * (CH + 1):
                                 (pm * NH + h + 1) * (CH + 1)]
                        for n in range(2):
                            nsl = slice(512 * n, 512 * (n + 1))
                            nc.tensor.matmul(pa_t[:, nsl], lhs, pe[:, nsl],
                                             start=(pm == 0), stop=False)
                    prev = (m, e_t)
                    if insert_after_m and m in insert_after_m:
                        insert_after_m[m]()
                pm, pe = prev
                lhs = vt[:, (pm * NH + h) * (CH + 1):
                         (pm * NH + h + 1) * (CH + 1)]
                for n in range(2):
                    nsl = slice(512 * n, 512 * (n + 1))
                    nc.tensor.matmul(pa_t[:, nsl], lhs, pe[:, nsl],
                                     start=False, stop=True)
                # normalize: 1/Z (fast approx) -> broadcast -> multiply
                # (reciprocal_approx_fast misreads PSUM in this context —
                # hop through SBUF partition 0 first)
                zt = z_p.tile([1, L], F32, tag="zt", name=f"zt{b}_{h}")
                nc.vector.tensor_copy(zt[:], pa_t[CH:CH + 1, :])
                rz = z_p.tile([1, L], F32, tag="rz", name=f"rz{b}_{h}")
                nc.vector.reciprocal_approx_fast(rz[:], zt[:])
                rzb = zb_p.tile([CH, L], F32, tag="zb", name=f"zb{b}_{h}")
                nc.gpsimd.partition_broadcast(rzb[:], rz[:])
                if h % 2 == 0:
                    a_tiles[h // 2] = a_p.tile([128, L], MM_DT, tag="a",
                                               name=f"a{b}_{h // 2}")
                rows = slice(CH * (h % 2), CH * (h % 2) + CH)
                nc.vector.tensor_mul(a_tiles[h // 2][rows, :],
                                     pa_t[0:CH, :], rzb[:])
                if insert_end:
                    insert_end()

            def proj(b, js=range(CT)):
                xs = st8[b]["xs"]
                a_tiles = st8[b]["a"]
                for j in js:
                    pp = ps_p.tile([128, 1024], F32, tag="mm", name=f"pp{b}_{j}")
                    for n in range(2):
                        nsl = slice(512 * n, 512 * (n + 1))
                        for i in range(CT):
                            nc.tensor.matmul(
                                pp[:, nsl],
                                wp[i][:, 128 * j:128 * (j + 1)],
                                a_tiles[i][:, nsl],
                                start=(i == 0), stop=(i == CT - 1))
                    o_t = out_p.tile([128, L], F32, tag="o", name=f"o{b}_{j}")
                    nc.vector.tensor_add(o_t[:], pp[:, :], xs[j][:])
                    nc.sync.dma_start(out_d[b, 128 * j:128 * (j + 1), :], o_t[:])

            def dump_stage(b, tiles):
                for i in range(CT):
                    nc.sync.dma_start(out_d[b, 128 * i:128 * (i + 1), 0:512],
                                      tiles[i][:].bitcast(F32))

            def warmup():
                # ~4us of full-array matmuls as soon as the weights land, so
                # the HAM clock gate reaches 8/8 before the real phases
                dummy = ps_p.tile([128, 1024], F32, tag="mm", name="dummy")
                for k in range(18):
                    nc.tensor.matmul(dummy[:, 0:512], wq[0][:, 0:128],
                                     wq[0][:, 0:512],
                                     start=(k == 0), stop=(k == 17))

            # ---------------- emission schedule ----------------
            # (gn_stats(1) must come AFTER gn(0)'s DVE ops: the DVE queue is
            # in-order and gn_stats(1) blocks on the x(1) DMA)
            gn_stats(0)
            gn(0)
            gn_stats(1)
            if STAGE == 1:
                dump_stage(0, st8[0]["xn"])
                gn(1)
                dump_stage(1, st8[1]["xn"])
            elif STAGE == 2:
                for mp in range(4):
                    vphase(0, mp)
                for j in range(8):
                    qkv(0, j)
                dump_stage(0, st8[0]["qk"][0:CT])
                gn(1)
                for mp in range(4):
                    vphase(1, mp)
                for j in range(8):
                    qkv(1, j)
                dump_stage(1, st8[1]["qk"][0:CT])
            else:
                # Sequential dense blocks; every attention block is entered
                # with the PE warm (preceded by full-array matmul phases) and
                # is never PE-idle >~3.4us inside, so the HAM clock gate
                # stays at 8/8.  Both GN phases run up-front (all Sqrt
                # activations precede the first Exp: one table load each).
                # (Phase insertions INTO attention were tried and are a wash:
                # the exp stream is saturated with no run-ahead buffering, so
                # any inserted PE work stalls it by its full duration.)
                gn(1)
                warmup()
                for mp in range(4):
                    vphase(0, mp)
                for j in range(8):
                    qkv(0, j)
                for h in range(NH):
                    attn_head(0, h)
                for mp in range(4):
                    vphase(1, mp)
                for j in range(8):
                    qkv(1, j)
                proj(0, js=(0, 1, 2))
                # densify the A(1) entry ramp with proj(0)'s last block
                attn_head(1, 0, insert_after_m={1: (lambda: proj(0, js=(3,)))})
                for h in range(1, NH):
                    attn_head(1, h)
                proj(1)

    nc.compile()
    return nc


_prog_cache = {}


def _get_program():
    if "prog" not in _prog_cache:
        _prog_cache["prog"] = _build_program()
    return _prog_cache["prog"]


def _host_constants():
    # group selector: sel[i][p, g] = 1 where global group of (tile i, part p)
    # is g;  fan[i][g, p] = same, transposed (for the fan-out matmul lhsT).
    sel = np.zeros((CT, 128, GROUPS), dtype=np.float32)
    fan = np.zeros((CT, GROUPS, 128), dtype=np.float32)
    for i in range(CT):
        for p in range(128):
            g = (128 * i + p) // GS
            sel[i, p, g] = 1.0
            fan[i, g, p] = 1.0
    return sel, fan


def kernel(x, norm_w, norm_b, qkv_w, qkv_b, proj_w, proj_b):
    global LAST_RESULTS
    x = np.ascontiguousarray(np.asarray(x, dtype=np.float32))
    np_mm = mybir.dt.np(MM_DT)
    qkv_w = np.asarray(qkv_w, dtype=np.float32)
    proj_w = np.asarray(proj_w, dtype=np.float32)
    qkv_b = np.ascontiguousarray(np.asarray(qkv_b, dtype=np.float32))
    proj_b = np.ascontiguousarray(np.asarray(proj_b, dtype=np.float32))
    wqkvT = np.ascontiguousarray(qkv_w.T.astype(np_mm))
    wprojT = np.ascontiguousarray(proj_w.T.astype(np_mm))
    # softmax rows sum to 1, so the v-bias contributes exactly
    # proj_w @ v_bias to the proj output; fold it plus proj_b into one
    # per-channel constant added at the residual.
    b_eff = np.ascontiguousarray(
        proj_w @ qkv_b[2 * C:3 * C] + proj_b).astype(np.float32)
    sel, fan = _host_constants()

    xr = x.reshape(B, C, L)
    nc = _get_program()

    common = {
        "wqkvT": wqkvT,
        "wprojT": wprojT,
        "norm_w": np.ascontiguousarray(norm_w, dtype=np.float32),
        "norm_b": np.ascontiguousarray(norm_b, dtype=np.float32),
        "qkv_b": qkv_b,
        "b_eff": b_eff,
        "sel": sel,
        "fan": fan,
    }
    in_maps = []
    for c in range(N_CORES):
        m = dict(common)
        m["x"] = np.ascontiguousarray(xr[BL * c:BL * (c + 1)])
        in_maps.append(m)

    trace = os.environ.get("KERNEL_TRACE", "0") == "1"
    kwargs = {}
    if trace:
        kwargs = dict(trace=True, trace_cores=[0])
    res = run_bass_kernel_spmd(nc, in_maps, core_ids=list(range(N_CORES)),
                               **kwargs)
    LAST_RESULTS = res
    out = np.concatenate([res.results[c]["out"] for c in range(N_CORES)], axis=0)
    return out.reshape(B, C, HH, WW)
